# revision 29
# baseline (speedup 1.0000x reference)
"""AdaptiveResonanceNetwork on 8 trn2 NeuronCores — hand-written Bass/Tile kernel.

Pure data parallelism: batch B=131072 split into 8 shards of 16384 rows, one
per NeuronCore; all parameters replicated (folded on host first). Each core
runs the full pipeline (3 encoders -> fusion -> 3 resonance layers -> SOFM
winner counts) and returns its [64] winner-count vector. Host sums counts,
forms the mean-pooled 192-vector (counts @ grid / B) and applies the tiny
192->6 output head.

Algebraic folding done on host (all exact, f64):
- LN mean-centering folded into the weight matrices (so feature-means are 0
  by construction); LN gain g folded into weights; variance computed as a
  weighted square-sum (weights 1/(H g^2)) so rstd applies to the g-scaled
  activations directly.
- Per-layer q/k projections folded into one matrix P_i = wq_i @ KblkT_i
  (scores = x @ P_i + pb_i, pre-scaled by 1/sqrt(hd)).
- V and the output projection folded into W_i = Vblk_i @ wo_i; output biases
  bo_i folded into the next layer's score bias.
- Layer-2 output projection folded into the SOFM distance matmul:
  argmin_g ||x - grid_g||^2 == argmax_g (attn2 @ (Vblk wo_2 grid^T) + cg).

On-chip layout: feature-major bf16 activations (chunks [128, *] + [64, *]),
f32 PSUM/statistics. SOFM winner selection via row-major scores from a
stationary-activation matmul, reduce_max + is_ge mask + mask^T @ ones counts.
"""
import os
import numpy as np
import ml_dtypes

import concourse.bass as bass
import concourse.tile as tile
from concourse import mybir, bacc
from concourse.bass_utils import run_bass_kernel_spmd

B_TOTAL = 131072
NCORES = 8
R_CORE = B_TOTAL // NCORES
H, NH, HD, MEM, GRID = 192, 4, 48, 16, 64
F32 = mybir.dt.float32
BF16 = mybir.dt.bfloat16
AF = mybir.ActivationFunctionType
OP = mybir.AluOpType
AX = mybir.AxisListType

ENC_KS = (("vib", 64), ("aco", 256), ("tmp", 128))

last_exec_time_ns = None
_cache = {}


# ---------------------------------------------------------------- host folding
def fold_params_np(p):
    out = {}

    def f32(x):
        return np.ascontiguousarray(np.asarray(x, np.float64).astype(np.float32))

    def bf16(x):
        return np.ascontiguousarray(
            np.asarray(x, np.float64).astype(ml_dtypes.bfloat16))

    for m, K in ENC_KS:
        W = np.asarray(p[f"enc_w_{m}"], np.float64)
        b = np.asarray(p[f"enc_b_{m}"], np.float64)
        g = np.asarray(p[f"enc_g_{m}"], np.float64)
        bb = np.asarray(p[f"enc_bb_{m}"], np.float64)
        Wc = W - W.mean(axis=1, keepdims=True)
        bc = b - b.mean()
        out[f"wp_{m}"] = bf16(Wc * g)
        bpv = (bc * g).reshape(H, 1)
        # variance weights reconstruct y^2 from the g-scaled activations;
        # clamp so g==0 features contribute 0 instead of NaN
        uv = (1.0 / (H * np.maximum(g * g, 1e-12))).reshape(H, 1)
        bbv = bb.reshape(H, 1)
        out[f"bp_{m}"] = f32(bpv)
        out[f"u_{m}"] = bf16(uv)
        out[f"bb_{m}"] = f32(bbv)
        out[f"bp2_{m}"] = f32(np.concatenate([bpv[128:], bpv[128:]]))
        out[f"u2_{m}"] = bf16(np.concatenate([uv[128:], uv[128:]]))
        out[f"bb2_{m}"] = f32(np.concatenate([bbv[128:], bbv[128:]]))

    Wf = np.asarray(p["fus_w"], np.float64)
    bf_ = np.asarray(p["fus_b"], np.float64)
    gf = np.asarray(p["fus_g"], np.float64)
    bbf = np.asarray(p["fus_bb"], np.float64)
    Wfc = (Wf - Wf.mean(axis=1, keepdims=True)) * gf
    bfc = (bf_ - bf_.mean()) * gf
    perm = np.concatenate([
        np.arange(0, 128), np.arange(192, 320), np.arange(384, 512),
        np.arange(128, 192), np.arange(320, 384), np.arange(512, 576)])
    out["wp_fusA"] = bf16(np.concatenate([Wfc[0:128], Wfc[192:320],
                                          Wfc[384:512]], axis=0))
    for nm2, rr in (("wfvB2", (128, 192)), ("wfaB2", (320, 384)),
                    ("wftB2", (512, 576))):
        cB = Wfc[rr[0]:rr[1]]
        out[nm2] = bf16(np.concatenate([cB, cB], axis=0))
    bpv = bfc.reshape(H, 1)
    uv = (1.0 / (H * np.maximum(gf * gf, 1e-12))).reshape(H, 1)
    bbv = bbf.reshape(H, 1)
    out["bp_fus"] = f32(bpv)
    out["u_fus"] = bf16(uv)
    out["bb_fus"] = f32(bbv)
    out["bp2_fus"] = f32(np.concatenate([bpv[128:], bpv[128:]]))
    out["u2_fus"] = bf16(np.concatenate([uv[128:], uv[128:]]))
    out["bb2_fus"] = f32(np.concatenate([bbv[128:], bbv[128:]]))

    scale = 1.0 / np.sqrt(HD)
    Ps, pbs, Ws, bos = [], [], [], []
    for i in range(3):
        mem = np.asarray(p["res_mem"][i], np.float64)
        k = (mem @ np.asarray(p["res_wk"][i], np.float64)
             + np.asarray(p["res_bk"][i], np.float64)).reshape(MEM, NH, HD)
        v = (mem @ np.asarray(p["res_wv"][i], np.float64)
             + np.asarray(p["res_bv"][i], np.float64)).reshape(MEM, NH, HD)
        KblkT = np.zeros((H, MEM * NH))
        Vblk = np.zeros((MEM * NH, H))
        for mm_ in range(MEM):
            for h in range(NH):
                KblkT[h * HD:(h + 1) * HD, mm_ * NH + h] = k[mm_, h]
                Vblk[mm_ * NH + h, h * HD:(h + 1) * HD] = v[mm_, h]
        Ps.append(np.asarray(p["res_wq"][i], np.float64) @ KblkT * scale)
        pbs.append(np.asarray(p["res_bq"][i], np.float64) @ KblkT * scale)
        Ws.append(Vblk @ np.asarray(p["res_wo"][i], np.float64))
        bos.append(np.asarray(p["res_bo"][i], np.float64))

    grid = np.asarray(p["grid"], np.float64)
    pb = [pbs[0], pbs[1] + bos[0] @ Ps[1], pbs[2] + bos[1] @ Ps[2]]
    VWG = Ws[2] @ grid.T
    cg = bos[2] @ grid.T - 0.5 * (grid ** 2).sum(axis=1)

    for i in range(3):
        out[f"P_{i}"] = bf16(Ps[i])
        out[f"p2d_{i}"] = bf16(np.concatenate([Ps[i][128:192], Ps[i][128:192]],
                                              axis=0))
        out[f"pb2_{i}"] = f32(np.concatenate([pb[i], pb[i]]).reshape(128, 1))
        out[f"pb1_{i}"] = f32(pb[i].reshape(64, 1))
    out["wi2_0"] = bf16(np.concatenate([Ws[0], Ws[0]], axis=0))
    out["wi2_1"] = bf16(np.concatenate([Ws[1], Ws[1]], axis=0))
    sel4 = np.zeros((4, 4 * 128))
    for j in range(4):
        sel4[j, j * 128:(j + 1) * 128] = 1.0
    out["sel4"] = bf16(sel4)
    selp = np.zeros((4, 2 * 128))
    for q in range(2):
        selp[2 * q, q * 128:q * 128 + 64] = 1.0
        selp[2 * q + 1, q * 128 + 64:q * 128 + 128] = 1.0
    out["selp"] = bf16(selp)
    out["vwg_aug"] = bf16(np.concatenate([VWG, cg.reshape(1, GRID)], axis=0))

    sumpat = np.zeros((128, 8))
    headpat = np.zeros((8, 128))
    for pe in range(128):
        s = (pe % NH) if pe < 64 else NH + ((pe - 64) % NH)
        sumpat[pe, s] = 1.0
        headpat[s, pe] = 1.0
    out["sumpat"] = bf16(sumpat)
    out["headpat"] = bf16(headpat)
    sumpat2 = np.zeros((64, 4))
    headpat2 = np.zeros((4, 64))
    for pe in range(64):
        sumpat2[pe, pe % NH] = 1.0
        headpat2[pe % NH, pe] = 1.0
    out["sumpat2"] = bf16(sumpat2)
    out["headpat2"] = bf16(headpat2)

    out["cnt_ones"] = bf16(np.ones((128, 1)))
    return out


PARAM_SPECS = {
    "wp_vib": ([64, H], BF16), "wp_aco": ([256, H], BF16),
    "wp_tmp": ([128, H], BF16), "wp_fusA": ([384, H], BF16),
    "wfvB2": ([128, H], BF16), "wfaB2": ([128, H], BF16),
    "wftB2": ([128, H], BF16),
    "bp_vib": ([H, 1], F32), "bp_aco": ([H, 1], F32), "bp_tmp": ([H, 1], F32),
    "bp_fus": ([H, 1], F32),
    "u_vib": ([H, 1], BF16), "u_aco": ([H, 1], BF16), "u_tmp": ([H, 1], BF16),
    "u_fus": ([H, 1], BF16),
    "bb_vib": ([H, 1], F32), "bb_aco": ([H, 1], F32), "bb_tmp": ([H, 1], F32),
    "bb_fus": ([H, 1], F32),
    "bp2_vib": ([128, 1], F32), "bp2_aco": ([128, 1], F32),
    "bp2_tmp": ([128, 1], F32), "bp2_fus": ([128, 1], F32),
    "u2_vib": ([128, 1], BF16), "u2_aco": ([128, 1], BF16),
    "u2_tmp": ([128, 1], BF16), "u2_fus": ([128, 1], BF16),
    "bb2_vib": ([128, 1], F32), "bb2_aco": ([128, 1], F32),
    "bb2_tmp": ([128, 1], F32), "bb2_fus": ([128, 1], F32),
    "selp": ([4, 2 * 128], BF16),
    "P_0": ([H, 64], BF16), "P_1": ([H, 64], BF16), "P_2": ([H, 64], BF16),
    "p2d_0": ([128, 64], BF16), "p2d_1": ([128, 64], BF16),
    "p2d_2": ([128, 64], BF16),
    "pb2_0": ([128, 1], F32), "pb2_1": ([128, 1], F32), "pb1_2": ([64, 1], F32),
    "wi2_0": ([128, H], BF16), "wi2_1": ([128, H], BF16),
    "sel4": ([4, 4 * 128], BF16),
    "vwg_aug": ([65, 64], BF16),
    "sumpat": ([128, 8], BF16), "headpat": ([8, 128], BF16),
    "sumpat2": ([64, 4], BF16), "headpat2": ([4, 64], BF16),
    "cnt_ones": ([128, 1], BF16),
}


# ---------------------------------------------------------------- nc builder
def _wsl(w, kc0, kc):
    if isinstance(w, list):
        c0, c, t = w[kc0 // 128]
        assert c0 == kc0 and c == kc
        return t[:]
    return w[kc0:kc0 + kc, :]


def _first128(w):
    if isinstance(w, list):
        return w[0][2][:]
    return w[0:128, :]


def _vecpair(w):
    if isinstance(w, list):
        return (w[0][2][:], w[1][2][:])
    return (w[0:128, :], w[128:192, :])


def _ln_layer(nc, tc, W, get_chunks, vp, out_a_ap, out_bp_ap, nb, GELU):
    """LN+gelu layer over nb blocks, B-chunks pair-packed on 128 partitions.

    vp: dict with bpA [128,1], bp2/u2/bb2 [128,1] (B-half duplicated),
    uA [128,1], bbA [128,1]. get_chunks(j) -> [(lw [K,192], rh, kb), ...]
    where lw[:,128:192] slices must be usable at row-base kb.
    out_a_ap(j) -> [128,512] target; out_bp_ap(p) -> [128,512] pair target.
    """
    with tc.tile_pool(name="ln_ps", bufs=4, space="PSUM") as ypool, \
         tc.tile_pool(name="ln_psb", bufs=2, space="PSUM") as ybpool, \
         tc.tile_pool(name="ln_stats", bufs=2, space="PSUM") as spool, \
         tc.tile_pool(name="ln_sb", bufs=1) as sbpool, \
         tc.tile_pool(name="ln_sq", bufs=4) as sqpool, \
         tc.tile_pool(name="ln_ss", bufs=4) as sspool, \
         tc.tile_pool(name="ln_sg", bufs=2) as sgpool, \
         tc.tile_pool(name="ln_rc", bufs=6) as rcpool, \
         tc.tile_pool(name="ln_r", bufs=8) as rpool:
        assert nb % 4 == 0
        y_a = sbpool.tile([128, nb, 512], BF16, tag="ysba")
        y_b = sbpool.tile([128, nb // 2, 512], BF16, tag="ysbb")

        ss1 = {}
        rstd_g = {}
        for jp0 in range(0, nb, 2):
            pr = jp0 // 2
            chlist = [get_chunks(jp0), get_chunks(jp0 + 1)]
            nch = len(chlist[0])
            yas = [ypool.tile([128, 512], F32, tag="ya", name=f"ya_{jp0}_{k}")
                   for k in range(2)]
            ybp = ybpool.tile([128, 512], F32, tag="ybp")
            sts = [spool.tile([1, 512], F32, tag="stats", name=f"st_{jp0}_{k}")
                   for k in range(2)]
            for ci in range(nch):
                for j2 in range(2):
                    lw, rh, kb = chlist[j2][ci]
                    nc.tensor.matmul(yas[j2][:], lw[:, 0:128], rh,
                                     start=(ci == 0), stop=(ci == nch - 1),
                                     tile_position=(kb, 0) if kb else None)
            for ci in range(nch):
                for j2 in range(2):
                    lw, rh, kb = chlist[j2][ci]
                    po = 64 * j2
                    nc.tensor.matmul(ybp[po:po + 64, :], lw[:, 128:192], rh,
                                     start=(ci == 0), stop=(ci == nch - 1),
                                     tile_position=(kb, po),
                                     skip_group_check=True)
            sq_bp = sqpool.tile([128, 512], BF16, tag="sqb")
            nc.scalar.activation(sq_bp[:], ybp[:], AF.Square, bias=vp["bp2"])
            nc.vector.tensor_scalar(out=y_b[:, pr, :], in0=ybp[:],
                                    scalar1=vp["bp2"], scalar2=None,
                                    op0=OP.add)
            for j2 in range(2):
                j = jp0 + j2
                ya, st = yas[j2], sts[j2]
                po = 64 * j2
                sqa = sqpool.tile([128, 512], BF16, tag="sqa")
                nc.scalar.activation(sqa[:], ya[:], AF.Square, bias=vp["bpA"])
                nc.vector.tensor_scalar(out=y_a[:, j, :], in0=ya[:],
                                        scalar1=vp["bpA"], scalar2=None,
                                        op0=OP.add)
                nc.tensor.matmul(st[:], vp["uA"], sqa[:],
                                 start=True, stop=False)
                nc.tensor.matmul(st[:], vp["u2"][po:po + 64, :],
                                 sq_bp[po:po + 64, :], start=False, stop=True,
                                 tile_position=(po, 0) if po else None)
                s1 = sspool.tile([1, 512], F32, tag="ss1")
                nc.vector.tensor_scalar(out=s1[:], in0=st[:], scalar1=1e-5,
                                        scalar2=None, op0=OP.add)
                ss1[j] = s1
                if j % 4 == 3:
                    g = j // 4
                    ssg = sgpool.tile([4, 512], F32, tag="ssg")
                    for k in range(4):
                        nc.gpsimd.dma_start(ssg[k:k + 1, :], ss1[4 * g + k][:])
                    rcg = rcpool.tile([4, 512], F32, tag="rcg")
                    nc.vector.reciprocal_approx_fast(rcg[:], ssg[:])
                    rstd_g[g] = rcg
        rg_all = {}
        for g in sorted(rstd_g):
            rg = rpool.tile([4, 512], BF16, tag="rstdg", name=f"rg_{g}")
            nc.scalar.activation(rg[:], rstd_g[g][:], AF.Sqrt)
            rg_all[g] = rg
        rstd_g = rg_all
        sel = W["sel4"]
        selp = W["selp"]
        # emit bc-MMs grouped by selector column so LDWEIGHTS amortizes
        for q in range(2):
            for pr in range(q, nb // 2, 2):
                rg = rstd_g[pr // 2]
                bcb = ybpool.tile([128, 512], F32, tag="ybp",
                                  name=f"bcb_{pr}")
                nc.tensor.matmul(bcb[:], selp[:, q * 128:(q + 1) * 128],
                                 rg[:], start=True, stop=True)
                zp_bp = sqpool.tile([128, 512], BF16, tag="sqb")
                nc.vector.tensor_tensor(out=zp_bp[:], in0=y_b[:, pr, :],
                                        in1=bcb[:], op=OP.mult)
                nc.scalar.activation(out_bp_ap(pr), zp_bp[:], GELU,
                                     bias=vp["bb2"])
        for jq in range(4):
            for g in range(nb // 4):
                j = 4 * g + jq
                rg = rstd_g[g]
                bca = ypool.tile([128, 512], F32, tag="ya", name=f"bca_{j}")
                nc.tensor.matmul(bca[:], sel[:, jq * 128:(jq + 1) * 128],
                                 rg[:], start=True, stop=True)
                zp_a = sqpool.tile([128, 512], BF16, tag="sqa")
                nc.vector.tensor_tensor(out=zp_a[:], in0=y_a[:, j, :],
                                        in1=bca[:], op=OP.mult)
                nc.scalar.activation(out_a_ap(j), zp_a[:], GELU,
                                     bias=vp["bbA"])


def _build_body(nc, tc, W, ins, out_counts, R, NBLK, n_super, NB_H, GELU):
    with tc.tile_pool(name="zbig", bufs=1) as zpool:
        zf_a = zpool.tile([128, NBLK, 512], BF16, tag="zra")
        zf_b = zpool.tile([128, NBLK // 2, 512], BF16, tag="zrb")

        with tc.tile_pool(name="zenc", bufs=1) as epool, \
             tc.tile_pool(name="xin", bufs=3) as xpool:
            z1 = epool.tile([128, NB_H, 512], BF16, tag="z1")
            z2 = epool.tile([128, NB_H, 512], BF16, tag="z2")
            z3 = epool.tile([128, NB_H, 512], BF16, tag="z3")
            zvB = epool.tile([128, NB_H // 2, 512], BF16, tag="zvB")
            zaB = epool.tile([128, NB_H // 2, 512], BF16, tag="zaB")
            ztB = epool.tile([128, NB_H // 2, 512], BF16, tag="ztB")

            for half in range(n_super):
                blk0 = half * NB_H

                def enc_layer(m, K, za, zbp):
                    wp = W[f"wp_{m}"]
                    x_dram = ins[f"x_{m}"]
                    SLAB = 4
                    slabs = {}

                    def chunks(j):
                        si = j // SLAB
                        if si not in slabs:
                            c0 = (blk0 + SLAB * si) * 512
                            cw = min(SLAB * 512, (NB_H - SLAB * si) * 512)
                            tiles = []
                            for kc0 in range(0, K, 128):
                                kc = min(128, K - kc0)
                                xt = xpool.tile([128, SLAB * 512], BF16,
                                                tag="xt")
                                nc.sync.dma_start(
                                    xt[0:kc, 0:cw],
                                    x_dram[kc0:kc0 + kc, c0:c0 + cw])
                                tiles.append((kc0, kc, xt))
                            slabs[si] = tiles
                        jo = j % SLAB
                        return [(_wsl(wp, kc0, kc),
                                 xt[0:kc, jo * 512:(jo + 1) * 512], 0)
                                for kc0, kc, xt in slabs[si]]

                    vp = {"bpA": _first128(W[f"bp_{m}"]),
                          "uA": _first128(W[f"u_{m}"]),
                          "bbA": _first128(W[f"bb_{m}"]),
                          "bp2": W[f"bp2_{m}"][:], "u2": W[f"u2_{m}"][:],
                          "bb2": W[f"bb2_{m}"][:]}
                    _ln_layer(nc, tc, W, chunks, vp,
                              lambda j: za[:, j, :],
                              lambda p: zbp[:, p, :],
                              NB_H, GELU)

                enc_layer("vib", 64, z1, zvB)
                enc_layer("aco", 256, z2, zaB)
                enc_layer("tmp", 128, z3, ztB)

                def fus_chunks(j):
                    wpa = W["wp_fusA"]
                    pj = 64 * (j % 2)
                    p = j // 2
                    return [
                        (_wsl(wpa, 0, 128), z1[:, j, :], 0),
                        (_wsl(wpa, 128, 128), z2[:, j, :], 0),
                        (_wsl(wpa, 256, 128), z3[:, j, :], 0),
                        (W["wfvB2"][pj:pj + 64, :], zvB[pj:pj + 64, p, :], pj),
                        (W["wfaB2"][pj:pj + 64, :], zaB[pj:pj + 64, p, :], pj),
                        (W["wftB2"][pj:pj + 64, :], ztB[pj:pj + 64, p, :], pj),
                    ]

                vpf = {"bpA": _first128(W["bp_fus"]),
                       "uA": _first128(W["u_fus"]),
                       "bbA": _first128(W["bb_fus"]),
                       "bp2": W["bp2_fus"][:], "u2": W["u2_fus"][:],
                       "bb2": W["bb2_fus"][:]}
                _ln_layer(nc, tc, W, fus_chunks, vpf,
                          lambda j, b0=blk0: zf_a[:, b0 + j, :],
                          lambda p, b0=blk0: zf_b[:, b0 // 2 + p, :],
                          NB_H, GELU)

        z_in_a, z_in_b = zf_a, zf_b
        zpool2_cm = tc.tile_pool(name="zbig2", bufs=1)
        zpool2 = zpool2_cm.__enter__()
        zpools = [zpool2, zpool]
        for i in range(2):
            P, Wi, P2d = W[f"P_{i}"], W[f"wi2_{i}"], W[f"p2d_{i}"]
            with tc.tile_pool(name="res_sc", bufs=2, space="PSUM") as scp, \
                 tc.tile_pool(name="res_sum", bufs=1, space="PSUM") as smp, \
                 tc.tile_pool(name="res_eb", bufs=1, space="PSUM") as ebp, \
                 tc.tile_pool(name="res_o", bufs=2, space="PSUM") as op_, \
                 tc.tile_pool(name="res_sb", bufs=1) as rsb, \
                 tc.tile_pool(name="res_wk", bufs=4) as rwk:
                zp = zpools[i]
                z_out_a = zp.tile([128, NBLK, 512], BF16, tag="zra")
                z_out_b = zp.tile([128, NBLK // 2, 512], BF16, tag="zrb")
                rs_sb = rsb.tile([8, NBLK // 2, 512], F32, tag="rssb")
                rs_bf = rsb.tile([8, NBLK // 2, 512], BF16, tag="rsbf")
                en_all = rsb.tile([128, NBLK // 2, 512], BF16, tag="enall")
                for prg in range(0, NBLK // 2, 2):
                    prs = [prg, prg + 1]
                    scs = {pr: scp.tile([128, 512], F32, tag="sc",
                                        name=f"sc_{i}_{pr}") for pr in prs}
                    for b01 in range(2):
                        po = 64 * b01
                        for pr in prs:
                            nc.tensor.matmul(scs[pr][po:po + 64, :],
                                             _wsl(P, 0, 128),
                                             z_in_a[:, 2 * pr + b01, :],
                                             start=True, stop=False,
                                             tile_position=(0, po),
                                             skip_group_check=True)
                        for pr in prs:
                            blk = 2 * pr + b01
                            nc.tensor.matmul(scs[pr][po:po + 64, :],
                                             P2d[po:po + 64, :],
                                             z_in_b[po:po + 64, blk // 2, :],
                                             start=False, stop=True,
                                             tile_position=(po, po),
                                             skip_group_check=True)
                    for pr in prs:
                        sc = scs[pr]
                        e_sb = rwk.tile([128, 512], BF16, tag="esb")
                        nc.scalar.activation(e_sb[:], sc[:], AF.Exp,
                                             bias=W[f"pb2_{i}"][:])
                        sm = smp.tile([8, 512], F32, tag="sm")
                        nc.tensor.matmul(sm[:], W["sumpat"][:], e_sb[:],
                                         start=True, stop=True)
                        nc.vector.reciprocal_approx_fast(rs_sb[:, pr, :],
                                                         sm[:])
                        nc.scalar.copy(rs_bf[:, pr, :], rs_sb[:, pr, :])
                        ebc = ebp.tile([128, 512], F32, tag="ebc")
                        nc.tensor.matmul(ebc[:], W["headpat"][:],
                                         rs_bf[:, pr, :],
                                         start=True, stop=True)
                        nc.vector.tensor_tensor(out=en_all[:, pr, :],
                                                in0=e_sb[:], in1=ebc[:],
                                                op=OP.mult)
                for pr in range(NBLK // 2):
                    obp = op_.tile([128, 512], F32, tag="obp")
                    for b01 in range(2):
                        blk = 2 * pr + b01
                        po = 64 * b01
                        oa = op_.tile([128, 512], F32, tag="oa")
                        nc.tensor.matmul(oa[:], Wi[po:po + 64, 0:128],
                                         en_all[po:po + 64, pr, :],
                                         start=True, stop=True,
                                         tile_position=(po, 0) if po else None)
                        nc.tensor.matmul(obp[po:po + 64, :],
                                         Wi[po:po + 64, 128:192],
                                         en_all[po:po + 64, pr, :],
                                         start=True, stop=True,
                                         tile_position=(po, po))
                        nc.vector.tensor_scalar(out=z_out_a[:, blk, :],
                                                in0=oa[:], scalar1=1.0,
                                                scalar2=None, op0=OP.mult)
                    nc.scalar.copy(z_out_b[:, pr, :], obp[:])
            z_in_a, z_in_b = z_out_a, z_out_b

        with tc.tile_pool(name="l2_sc", bufs=2, space="PSUM") as scp, \
             tc.tile_pool(name="l2_sum", bufs=1, space="PSUM") as smp, \
             tc.tile_pool(name="l2_r", bufs=2, space="PSUM") as rp, \
             tc.tile_pool(name="l2_cnt", bufs=1, space="PSUM") as cp, \
             tc.tile_pool(name="l2_sb", bufs=4) as sb2, \
             tc.tile_pool(name="l2_msk", bufs=4) as mp2:
            counts_ps = cp.tile([GRID, 1], F32, tag="cnt")
            nmm = 0
            for pr in range(NBLK // 2):
                r_ps = rp.tile([128, 8, 64], F32, tag="rps")
                for b01 in range(2):
                    blk = 2 * pr + b01
                    sc = scp.tile([64, 512], F32, tag="sc2")
                    pj = 64 * (blk % 2)
                    nc.tensor.matmul(sc[:], _wsl(W["P_2"], 0, 128),
                                     z_in_a[:, blk, :], start=True, stop=False)
                    nc.tensor.matmul(sc[:], W["p2d_2"][pj:pj + 64, :],
                                     z_in_b[pj:pj + 64, blk // 2, :],
                                     start=False, stop=True,
                                     tile_position=(pj, 0) if pj else None)
                    e2 = sb2.tile([65, 512], BF16, tag="e2")
                    nc.scalar.activation(e2[0:64, :], sc[:], AF.Exp,
                                         bias=W["pb1_2"][:])
                    sm = smp.tile([4, 512], F32, tag="sm2")
                    nc.tensor.matmul(sm[:], W["sumpat2"][:], e2[0:64, :],
                                     start=True, stop=True)
                    rs2 = sb2.tile([4, 512], F32, tag="rs2")
                    nc.vector.reciprocal_approx_fast(rs2[:], sm[:])
                    rs2b = sb2.tile([4, 512], BF16, tag="rs2b")
                    nc.scalar.copy(rs2b[:], rs2[:])
                    ebc = smp.tile([64, 512], F32, tag="ebc2")
                    nc.tensor.matmul(ebc[:], W["headpat2"][:], rs2b[:],
                                     start=True, stop=True)
                    en2 = sb2.tile([65, 512], BF16, tag="en2")
                    nc.vector.tensor_tensor(out=en2[0:64, :], in0=e2[0:64, :],
                                            in1=ebc[:], op=OP.mult)
                    nc.vector.memset(en2[64:65, :], 1.0)
                    for q in range(4):
                        nc.tensor.matmul(r_ps[:, 4 * b01 + q, :],
                                         en2[:, 128 * q:128 * (q + 1)],
                                         W["vwg_aug"][:], start=True, stop=True)
                mx = mp2.tile([128, 8], F32, tag="mx")
                nc.vector.tensor_reduce(mx[:], r_ps[:], axis=AX.X, op=OP.max)
                mask = mp2.tile([128, 8, 64], BF16, tag="mask")
                nc.vector.tensor_tensor(
                    out=mask[:], in0=r_ps[:],
                    in1=mx[:].unsqueeze(2).broadcast_to([128, 8, 64]),
                    op=OP.is_ge)
                for q8 in range(8):
                    nc.tensor.matmul(counts_ps[:], mask[:, q8, :],
                                     W["cnt_ones"][:], start=(nmm == 0),
                                     stop=(nmm == NBLK * 4 - 1))
                    nmm += 1
            counts_sb = sb2.tile([GRID, 1], F32, tag="csb")
            nc.vector.tensor_copy(counts_sb[:], counts_ps[:])
            nc.sync.dma_start(out_counts[:], counts_sb[:])
        zpool2_cm.__exit__(None, None, None)


def build_arn_nc(R=R_CORE, sim_safe=False, n_super=None):
    NBLK = R // 512
    assert R % 1024 == 0 and NBLK >= 2
    if n_super is None:
        n_super = 2 if NBLK > 16 else 1
    NB_H = NBLK // n_super
    assert NB_H <= 32
    GELU = AF.Tanh if sim_safe else AF.Gelu

    nc = bacc.Bacc()
    ins = {}
    for m, K in ENC_KS:
        ins[f"x_{m}"] = nc.declare_dram_parameter(f"x_{m}", [K, R], BF16,
                                                  isOutput=False)
    for nm, (shape, dt) in PARAM_SPECS.items():
        ins[nm] = nc.declare_dram_parameter(nm, shape, dt, isOutput=False)
    out_counts = nc.declare_dram_parameter("counts", [GRID, 1], F32,
                                           isOutput=True)

    with tile.TileContext(nc) as tc:
        with tc.tile_pool(name="weights", bufs=1) as wpool:
            W = {}
            for nm, (shape, dt) in PARAM_SPECS.items():
                K = shape[0]
                if K <= 128:
                    t = wpool.tile(list(shape), dt, tag=nm)
                    nc.sync.dma_start(t[:], ins[nm][:])
                    W[nm] = t
                else:
                    chunks = []
                    for kc0 in range(0, K, 128):
                        kc = min(128, K - kc0)
                        t = wpool.tile([kc] + list(shape[1:]), dt,
                                       tag=f"{nm}_{kc0}")
                        nc.sync.dma_start(t[:], ins[nm][kc0:kc0 + kc])
                        chunks.append((kc0, kc, t))
                    W[nm] = chunks

            _build_body(nc, tc, W, ins, out_counts, R, NBLK, n_super, NB_H,
                        GELU)
    nc.compile()
    return nc


# ---------------------------------------------------------------- entry point
def _head(pooled, out_w, out_b):
    out = pooled @ out_w + out_b
    sig = 1.0 / (1.0 + np.exp(-out))
    return np.stack(
        [sig[0], max(out[1], 0.0), sig[2], sig[3], sig[4], sig[5]]
    ).astype(np.float32)


def kernel(**inputs):
    global last_exec_time_ns
    folded = fold_params_np(inputs)
    params = {nm: folded[nm] for nm in PARAM_SPECS}

    xs = {}
    for m, K in ENC_KS:
        x = np.asarray(inputs[f"x_{m}"], np.float32).reshape(NCORES, R_CORE, K)
        xb = x.astype(ml_dtypes.bfloat16)
        xs[f"x_{m}"] = np.ascontiguousarray(xb.transpose(0, 2, 1))

    if "nc" not in _cache:
        _cache["nc"] = build_arn_nc(R=R_CORE, sim_safe=False)
    nc = _cache["nc"]

    in_maps = [
        {**params, **{k: np.ascontiguousarray(v[c]) for k, v in xs.items()}}
        for c in range(NCORES)
    ]
    trace = bool(int(os.environ.get("ARN_TRACE", "0")))
    res = run_bass_kernel_spmd(nc, in_maps, core_ids=list(range(NCORES)),
                               trace=trace)
    last_exec_time_ns = getattr(res, "exec_time_ns", None)
    counts = np.zeros(GRID, np.float64)
    for c in range(NCORES):
        counts += np.asarray(res.results[c]["counts"], np.float64).ravel()

    grid = np.asarray(inputs["grid"], np.float64)
    pooled = counts @ grid / float(B_TOTAL)
    return _head(pooled,
                 np.asarray(inputs["out_w"], np.float64),
                 np.asarray(inputs["out_b"], np.float64))
